# revision 16
# baseline (speedup 1.0000x reference)
"""ChildSum TreeLSTM on 8 Trainium2 NeuronCores.

Data-parallel over nodes with level-synchronous sparse evaluation:
  - Only nodes at level l update at step l. Each level's nodes are split
    across 8 cores; any (parent, child) edge with level gap <= 3 is forced
    onto one core ("co-location") so cross-core values are only needed
    >= 4 levels after they are produced.
  - Cross-core exchange: one AllGather per 2-level window (7 total),
    overlapped with compute thanks to the slack from co-location.
  - Per level each core indirect-gathers its nodes' children h/c rows
    (bf16) from a DRAM log, computes gates with bf16 matmuls (fp32 PSUM),
    and writes fresh state back.
  - Host->device traffic is minimized (the axon tunnel is the wall-clock
    bottleneck): embeddings are gathered+transposed on host (0.9 MB/core
    instead of the 19.2 MB table), the 8 weight matrices are sharded 8-way
    and AllGathered on device, the masked child-sum selector matrices are
    generated on device from compact int32 column indices, and state I/O
    is packed bf16.
"""

import numpy as np
import ml_dtypes

N, K, NLEV, V, IN, H, C = 8192, 6, 16, 32000, 300, 512, 8
D_COLOC = 4
HG = H // 128

_CACHE = {}


def _host_prep(input_ids, child_idx, child_mask, node_level):
    lvl = node_level.astype(np.int64)
    p_all = np.repeat(np.arange(N), K)
    c_all = child_idx.reshape(-1).astype(np.int64)
    valid = (child_mask.reshape(-1) != 0) & (lvl[c_all] < lvl[p_all])

    con = valid & (lvl[p_all] - lvl[c_all] <= D_COLOC - 1)
    uf = np.arange(N)

    def find(x):
        while uf[x] != x:
            uf[x] = uf[uf[x]]
            x = uf[x]
        return x

    for a, b in zip(p_all[con], c_all[con]):
        ra, rb = find(a), find(b)
        if ra != rb:
            uf[ra] = rb
    roots = np.array([find(i) for i in range(N)])

    comp = {}
    for i, r in enumerate(roots):
        comp.setdefault(r, []).append(i)
    comps = sorted(comp.values(), key=len, reverse=True)
    core_lvl = np.zeros((C, NLEV), np.int64)
    core_of = np.zeros(N, np.int64)
    for nodes in comps:
        clv = np.bincount(lvl[np.array(nodes)], minlength=NLEV)
        best = min(range(C),
                   key=lambda c: (int(np.max(core_lvl[c] + clv)),
                                  int(core_lvl[c].sum())))
        core_lvl[best] += clv
        for n in nodes:
            core_of[n] = best

    PAD = core_lvl.max(axis=0).astype(int)
    assert PAD.max() <= 128, f"PAD overflow {PAD}"
    S = np.concatenate([[0], np.cumsum(PAD)]).astype(int)  # level row start
    NR = int(S[NLEV])
    NBX = (NR + 127) // 128

    nodes_cl = [[np.where((core_of == c) & (lvl == l))[0]
                 for l in range(NLEV)] for c in range(C)]
    slot_of = np.zeros(N, np.int64)
    for c in range(C):
        for l in range(NLEV):
            slot_of[nodes_cl[c][l]] = np.arange(len(nodes_cl[c][l]))

    NW = max((NLEV - 2) // 2, 0)
    WROWS = [int(PAD[2 * w] + PAD[2 * w + 1]) for w in range(NW)]
    winbase = [0] * NW
    r = 1
    for w in range(NW):
        winbase[w] = r
        r += C * WROWS[w]
    own_base = r
    RL = own_base + NLEV * 128

    def log_row(c_req, child):
        o, lam, j = core_of[child], lvl[child], slot_of[child]
        if o == c_req:
            return own_base + lam * 128 + j
        w = lam // 2
        assert w < NW
        off = (PAD[2 * w] if lam == 2 * w + 1 else 0) + j
        return winbase[w] + o * WROWS[w] + off

    meta_lvl = []
    per_core = {c: {"gi": [], "oi": [], "pc": []} for c in range(C)}
    glo = pco = 0
    for l in range(NLEV):
        info = []
        for c in range(C):
            logs, owns = [], []
            for n in nodes_cl[c][l]:
                j = slot_of[n]
                for k in range(K):
                    if not valid[n * K + k]:
                        continue
                    ch = c_all[n * K + k]
                    if core_of[ch] == c:
                        owns.append((j, log_row(c, ch)))
                    else:
                        logs.append((j, log_row(c, ch)))
            info.append((logs, owns))
        n_log = max(len(i[0]) for i in info)
        n_own = max(len(i[1]) for i in info)
        g_log = (n_log + 127) // 128
        assert n_own <= 128, (l, n_own)
        g_self = 1 if n_own > 0 else 0
        G = g_log + g_self
        meta_lvl.append((int(PAD[l]), g_log, g_self, G, int(S[l]), glo, pco))
        for c in range(C):
            logs, owns = info[c]
            gi = np.zeros((128, max(g_log, 1)), np.int32)
            pc_m = np.full((128, max(G, 1)), -1, np.int32)
            for r_i, (j, row) in enumerate(logs):
                gi[r_i % 128, r_i // 128] = 2 * row
                pc_m[r_i % 128, r_i // 128] = j
            oi = np.zeros((128, 1), np.int32)
            for r_i, (j, row) in enumerate(owns):
                oi[r_i, 0] = 2 * row
                pc_m[r_i, g_log] = j
            pcd = per_core[c]
            pcd["gi"].append(gi)
            pcd["oi"].append(oi)
            pcd["pc"].append(pc_m)
        glo += g_log
        pco += G
    GIC, PCC = max(glo, 1), max(pco, 1)  # packed gidx / pcol column counts
    return (core_of, nodes_cl, PAD, S, NR, NBX, NW, WROWS,
            winbase, own_base, RL, meta_lvl, per_core, GIC, PCC)


def kernel(**inputs):
    from concourse.bass_utils import run_bass_kernel_spmd
    nc, in_maps, assemble = _prepare(**inputs)
    res = run_bass_kernel_spmd(nc, in_maps, list(range(C)))
    return assemble([r for r in res.results])


def _prepare(input_ids, child_idx, child_mask, node_level, num_levels, emb,
             W_ix, b_ix, W_ih, b_ih, W_fx, b_fx, W_fh, b_fh,
             W_ox, b_ox, W_oh, b_oh, W_ux, b_ux, W_uh, b_uh):
    input_ids = np.asarray(input_ids)
    child_idx = np.asarray(child_idx)
    child_mask = np.asarray(child_mask)
    node_level = np.asarray(node_level)
    emb = np.asarray(emb, dtype=np.float32)
    assert int(num_levels) == NLEV

    (core_of, nodes_cl, PAD, S, NR, NBX, NW, WROWS, winbase,
     own_base, RL, meta_lvl, per_core, GIC, PCC) = _host_prep(
        input_ids, child_idx, child_mask, node_level)

    bf16 = ml_dtypes.bfloat16
    Wx = np.zeros((384, 4 * H), np.float32)
    Wx[:IN, 0 * H:1 * H] = np.asarray(W_ix)
    Wx[:IN, 1 * H:2 * H] = np.asarray(W_ox)
    Wx[:IN, 2 * H:3 * H] = np.asarray(W_ux)
    Wx[:IN, 3 * H:4 * H] = np.asarray(W_fx)
    Wx[320, 0 * H:1 * H] = np.asarray(b_ix) + np.asarray(b_ih)
    Wx[320, 1 * H:2 * H] = np.asarray(b_ox) + np.asarray(b_oh)
    Wx[320, 2 * H:3 * H] = np.asarray(b_ux) + np.asarray(b_uh)
    Wx[320, 3 * H:4 * H] = np.asarray(b_fx) + np.asarray(b_fh)
    Whiou = np.concatenate(
        [np.asarray(W_ih), np.asarray(W_oh), np.asarray(W_uh)],
        axis=1).astype(bf16)
    Wfh = np.asarray(W_fh).astype(bf16)
    Wx = Wx.astype(bf16)

    XC = NBX * 128
    WXR, WHR, WFR = 384 // C, H // C, H // C

    in_maps = []
    for c in range(C):
        pcd = per_core[c]
        # host-side embedding gather + transpose into matmul lhsT layout:
        # xT[p, kk, S[l]+j] = x_node[kk*128+p]; bias row at (kk=2, p=64)
        xT = np.zeros((128, 3, NR), bf16)
        for l in range(NLEV):
            nn = nodes_cl[c][l]
            if len(nn) == 0:
                continue
            xr = emb[input_ids[nn]].astype(bf16)        # [n_l, IN]
            cols = S[l] + np.arange(len(nn))
            xT[:, 0, cols] = xr[:, 0:128].T
            xT[:, 1, cols] = xr[:, 128:256].T
            xT[:IN - 256, 2, cols] = xr[:, 256:IN].T
        xT[64, 2, :] = 1.0                              # bias multiplier
        gi = np.zeros((128, GIC), np.int32)
        oi = np.zeros((128, NLEV), np.int32)
        pcol = np.full((128, PCC), -1, np.float32)
        for l in range(NLEV):
            PADl, g_log, g_self, G, S_l, glo, pco = meta_lvl[l]
            g = pcd["gi"][l]
            gi[:, glo:glo + g_log] = g[:, :g_log]
            oi[:, l] = pcd["oi"][l][:, 0]
            pcol[:, pco:pco + G] = pcd["pc"][l][:, :G].astype(np.float32)
        in_maps.append({
            "xT": np.ascontiguousarray(xT.reshape(128, -1)),
            "WxS": np.ascontiguousarray(Wx[c * WXR:(c + 1) * WXR]),
            "WhiouS": np.ascontiguousarray(Whiou[c * WHR:(c + 1) * WHR]),
            "WfhS": np.ascontiguousarray(Wfh[c * WFR:(c + 1) * WFR]),
            "gidx": gi,
            "oidx": oi,
            "pcol": pcol,
        })

    import os
    key = (tuple(meta_lvl), NBX, NR, RL, tuple(WROWS), GIC, PCC,
           os.environ.get("KERNEL_NO_CC", ""),
           os.environ.get("KERNEL_CC_MAX", ""),
           os.environ.get("KERNEL_CC_SEP", ""),
           os.environ.get("KERNEL_CC_NOSHARED", ""))
    if key not in _CACHE:
        _CACHE[key] = _build(key)
    nc = _CACHE[key]

    def assemble(results):
        out = np.zeros((N, H), np.float32)
        for c in range(C):
            oh = (np.asarray(results[c]["out_h"]).astype(np.float32) - 128.0)
            oh *= np.asarray(results[c]["out_s"]).astype(np.float32)
            for l in range(NLEV):
                nn = nodes_cl[c][l]
                out[nn] = oh[S[l]:S[l] + len(nn)]
        return out

    return nc, in_maps, assemble


def _build(key):
    import concourse.bass as bass
    import concourse.bacc as bacc
    import concourse.mybir as mybir
    import concourse.tile as tile
    from concourse.masks import make_identity
    from contextlib import ExitStack

    meta_lvl, NBX, NR, RL, WROWS, GIC, PCC = key[:7]
    import os
    NO_CC = bool(os.environ.get("KERNEL_NO_CC", ""))
    CC_MAX = int(os.environ.get("KERNEL_CC_MAX", "99"))
    NOSHARED = bool(os.environ.get("KERNEL_CC_NOSHARED", ""))
    CC_SEP = bool(os.environ.get("KERNEL_CC_SEP", ""))
    meta_lvl = list(meta_lvl)
    NW = len(WROWS)
    XC = NBX * 128
    Gmax = max(max(m[3] for m in meta_lvl), 1)
    dt = mybir.dt
    f32, bf, i32 = dt.float32, dt.bfloat16, dt.int32
    i8 = dt.int8
    SIG = mybir.ActivationFunctionType.Sigmoid
    TANH = mybir.ActivationFunctionType.Tanh
    RECIP = mybir.ActivationFunctionType.Reciprocal
    WXR, WHR, WFR = 384 // C, H // C, H // C

    winbase = [0] * NW
    r = 1
    for w in range(NW):
        winbase[w] = r
        r += C * WROWS[w]
    own_base = r

    nc = bacc.Bacc("TRN2", target_bir_lowering=False, debug=False,
                   num_devices=C)
    T_xT = nc.dram_tensor("xT", [128, 3 * NR], bf, kind="ExternalInput")
    T_WxS = nc.dram_tensor("WxS", [WXR, 4 * H], bf, kind="ExternalInput")
    T_WhS = nc.dram_tensor("WhiouS", [WHR, 3 * H], bf, kind="ExternalInput")
    T_WfS = nc.dram_tensor("WfhS", [WFR, H], bf, kind="ExternalInput")
    T_gidx = nc.dram_tensor("gidx", [128, GIC], i32, kind="ExternalInput")
    T_oidx = nc.dram_tensor("oidx", [128, NLEV], i32, kind="ExternalInput")
    T_pcol = nc.dram_tensor("pcol", [128, PCC], f32, kind="ExternalInput")
    u8 = dt.uint8
    T_out = nc.dram_tensor("out_h", [NR, H], u8, kind="ExternalOutput")
    T_os = nc.dram_tensor("out_s", [NR, 1], f32, kind="ExternalOutput")

    T_log = nc.dram_tensor("log", [2 * RL, H], bf)
    T_xg = nc.dram_tensor("xg", [XC, 4 * H], f32)
    T_ccin = [nc.dram_tensor(f"ccin{w}", [WROWS[w], 2 * H], bf)
              for w in range(NW)]
    kw = {} if NOSHARED else {"addr_space": "Shared"}
    T_WxG = nc.dram_tensor("WxG", [384, 4 * H], bf, **kw)
    T_WhG = nc.dram_tensor("WhG", [H, 3 * H], bf, **kw)
    T_WfG = nc.dram_tensor("WfG", [H, H], bf, **kw)
    # collectives may not read IO tensors; bounce input shards to Internal
    T_WxB = nc.dram_tensor("WxB", [WXR, 4 * H], bf)
    T_WhB = nc.dram_tensor("WhB", [WHR, 3 * H], bf)
    T_WfB = nc.dram_tensor("WfB", [WFR, H], bf)
    if CC_SEP:
        T_ccout = [nc.dram_tensor(f"ccout{w}", [C * WROWS[w], 2 * H], bf,
                                  **kw) for w in range(NW)]
    else:
        wmax = max(WROWS)
        _rot = [nc.dram_tensor(f"ccout{i}", [C * wmax, 2 * H], bf, **kw)
                for i in range(min(3, NW))]
        T_ccout = [_rot[w % 3] for w in range(NW)]

    with tile.TileContext(nc) as tc, ExitStack() as ctx:
        wpool = ctx.enter_context(tc.tile_pool(name="weights", bufs=1))
        sp = ctx.enter_context(tc.tile_pool(name="spsum", bufs=3,
                                            space="PSUM"))
        bp = ctx.enter_context(tc.tile_pool(name="bpsum", bufs=3,
                                            space="PSUM"))
        cnp = ctx.enter_context(tc.tile_pool(name="cnpsum", bufs=2,
                                             space="PSUM"))
        work = ctx.enter_context(tc.tile_pool(name="work", bufs=3))
        gates = ctx.enter_context(tc.tile_pool(name="gates", bufs=2))
        dpool = ctx.enter_context(tc.tile_pool(name="delta", bufs=3))

        # ---- gather the 8-way-sharded weights across cores
        nc.sync.dma_start(out=T_WxB[:], in_=T_WxS[:])
        nc.sync.dma_start(out=T_WhB[:], in_=T_WhS[:])
        nc.sync.dma_start(out=T_WfB[:], in_=T_WfS[:])
        nc.gpsimd.collective_compute(
            "AllGather", mybir.AluOpType.bypass,
            replica_groups=[list(range(C))],
            ins=[T_WxB[:]], outs=[T_WxG[:]])
        nc.gpsimd.collective_compute(
            "AllGather", mybir.AluOpType.bypass,
            replica_groups=[list(range(C))],
            ins=[T_WhB[:]], outs=[T_WhG[:]])
        nc.gpsimd.collective_compute(
            "AllGather", mybir.AluOpType.bypass,
            replica_groups=[list(range(C))],
            ins=[T_WfB[:]], outs=[T_WfG[:]])

        ident = wpool.tile([128, 128], bf)
        make_identity(nc, ident[:])
        w_x = wpool.tile([128, 3, 4 * H], bf)
        nc.sync.dma_start(out=w_x[:], in_=T_WxG[:].rearrange(
            "(t p) n -> p t n", p=128))
        w_iou = wpool.tile([128, HG, 3 * H], bf)
        nc.sync.dma_start(out=w_iou[:], in_=T_WhG[:].rearrange(
            "(t p) n -> p t n", p=128))
        w_fh = wpool.tile([128, HG, H], bf)
        nc.sync.dma_start(out=w_fh[:], in_=T_WfG[:].rearrange(
            "(t p) n -> p t n", p=128))
        t_gidx = wpool.tile([128, GIC], i32)
        nc.sync.dma_start(out=t_gidx[:], in_=T_gidx[:])
        t_oidx = wpool.tile([128, NLEV], i32)
        nc.sync.dma_start(out=t_oidx[:], in_=T_oidx[:])
        t_pcol = wpool.tile([128, PCC], f32)
        nc.sync.dma_start(out=t_pcol[:], in_=T_pcol[:])
        t_xT = wpool.tile([128, 3 * NR], bf)
        nc.sync.dma_start(out=t_xT[:], in_=T_xT[:])

        zrow = wpool.tile([2, H], bf)
        nc.gpsimd.memset(zrow[:], 0.0)
        nc.sync.dma_start(out=T_log[0:2, :], in_=zrow[:])

        # ---- generate P (child->parent-slot selector) and its transpose
        # from packed column indices: P[p, g*128+j] = (pcol[p, g] == j)
        t_iota = wpool.tile([128, 128], f32)
        nc.gpsimd.iota(t_iota[:], pattern=[[1, 128]], base=0,
                       channel_multiplier=0,
                       allow_small_or_imprecise_dtypes=True)
        t_P = wpool.tile([128, PCC * 128], bf)
        t_PT = wpool.tile([128, PCC * 128], bf)
        for g in range(PCC):
            eng = (nc.vector, nc.gpsimd)[g % 2]
            eng.tensor_scalar(
                out=t_P[:, g * 128:(g + 1) * 128], in0=t_iota[:],
                scalar1=t_pcol[:, g:g + 1], scalar2=None,
                op0=mybir.AluOpType.is_equal)
            tp = sp.tile([128, 128], bf, tag="sp", space="PSUM")
            nc.tensor.transpose(out=tp[:], in_=t_P[:, g * 128:(g + 1) * 128],
                                identity=ident[:])
            if g % 2 == 0:
                nc.vector.tensor_copy(t_PT[:, g * 128:(g + 1) * 128], tp[:])
            else:
                nc.scalar.copy(t_PT[:, g * 128:(g + 1) * 128], tp[:])

        # ---- phase 1: x projections (xT is host-gathered + transposed)
        for b in range(NBX):
            cb = min(128, NR - b * 128)
            xg_sb = work.tile([128, 4 * H], f32, tag="xg")
            for nb4 in range(4):
                px = bp.tile([128, H], f32, tag="bp", space="PSUM")
                for kk in range(3):
                    nc.tensor.matmul(
                        px[:cb, :],
                        lhsT=t_xT[:, kk * NR + b * 128:kk * NR + b * 128 + cb],
                        rhs=w_x[:, kk, nb4 * H:(nb4 + 1) * H],
                        start=(kk == 0), stop=(kk == 2))
                dst = xg_sb[:cb, nb4 * H:(nb4 + 1) * H]
                if nb4 % 2 == 0:
                    nc.vector.tensor_copy(dst, px[:cb, :])
                else:
                    nc.scalar.copy(dst, px[:cb, :])
            nc.sync.dma_start(out=T_xg[b * 128:b * 128 + cb, :],
                              in_=xg_sb[:cb, :])

        # ---- level loop
        for l in range(NLEV):
            PADl, g_log, g_self, G, S_l, glo, pco = meta_lvl[l]
            w_id = l // 2
            poff = pco * 128

            xg_l = work.tile([128, 4 * H], f32, tag="xg")
            nc.sync.dma_start(out=xg_l[:PADl, :],
                              in_=T_xg[S_l:S_l + PADl, :])

            if G > 0:
                ch = work.tile([128, Gmax, H], bf, tag="ch")
                cc = work.tile([128, Gmax, H], bf, tag="cc")
                for g in range(g_log):
                    nc.gpsimd.indirect_dma_start(
                        out=ch[:, g, :], out_offset=None,
                        in_=T_log[:],
                        in_offset=bass.IndirectOffsetOnAxis(
                            ap=t_gidx[:, glo + g:glo + g + 1], axis=0))
                    nc.gpsimd.indirect_dma_start(
                        out=cc[:, g, :], out_offset=None,
                        in_=T_log[:], element_offset=H,
                        in_offset=bass.IndirectOffsetOnAxis(
                            ap=t_gidx[:, glo + g:glo + g + 1], axis=0))
                if g_self > 0:
                    nc.gpsimd.indirect_dma_start(
                        out=ch[:, g_log, :], out_offset=None, in_=T_log[:],
                        in_offset=bass.IndirectOffsetOnAxis(
                            ap=t_oidx[:, l:l + 1], axis=0))
                    nc.gpsimd.indirect_dma_start(
                        out=cc[:, g_log, :], out_offset=None, in_=T_log[:],
                        element_offset=H,
                        in_offset=bass.IndirectOffsetOnAxis(
                            ap=t_oidx[:, l:l + 1], axis=0))

                chT = work.tile([128, HG, Gmax * 128], bf, tag="chT")
                for g in range(G):
                    for kk in range(HG):
                        tp = sp.tile([128, 128], bf, tag="sp", space="PSUM")
                        nc.tensor.transpose(
                            out=tp[:],
                            in_=ch[:, g, kk * 128:(kk + 1) * 128],
                            identity=ident[:])
                        dst = chT[:, kk, g * 128:(g + 1) * 128]
                        if (g + kk) % 2 == 0:
                            nc.vector.tensor_copy(dst, tp[:])
                        else:
                            nc.scalar.copy(dst, tp[:])

                hsT = gates.tile([128, HG, 128], bf, tag="hsT")
                for kk in range(HG):
                    ps = sp.tile([128, 128], f32, tag="sp", space="PSUM")
                    for g in range(G):
                        nc.tensor.matmul(
                            ps[:, :PADl],
                            lhsT=ch[:, g, kk * 128:(kk + 1) * 128],
                            rhs=t_P[:, poff + g * 128:poff + g * 128 + PADl],
                            start=(g == 0), stop=(g == G - 1))
                    nc.vector.tensor_copy(hsT[:, kk, :PADl], ps[:, :PADl])

            i_t = gates.tile([128, H], f32, tag="i")
            o_t = gates.tile([128, H], f32, tag="o")
            u_t = gates.tile([128, H], f32, tag="u")
            for nb3, dst in ((0, i_t), (1, o_t), (2, u_t)):
                fn = TANH if nb3 == 2 else SIG
                if G > 0:
                    pg = bp.tile([128, H], f32, tag="bp", space="PSUM")
                    for kk in range(HG):
                        nc.tensor.matmul(
                            pg[:PADl, :], lhsT=hsT[:, kk, :PADl],
                            rhs=w_iou[:, kk, nb3 * H:(nb3 + 1) * H],
                            start=(kk == 0), stop=(kk == HG - 1))
                    pre = gates.tile([128, H], f32, tag="pre")
                    nc.vector.tensor_tensor(
                        pre[:PADl, :], pg[:PADl, :],
                        xg_l[:PADl, nb3 * H:(nb3 + 1) * H],
                        op=mybir.AluOpType.add)
                    nc.scalar.activation(dst[:PADl, :], pre[:PADl, :], fn)
                else:
                    nc.scalar.activation(
                        dst[:PADl, :], xg_l[:PADl, nb3 * H:(nb3 + 1) * H],
                        fn)

            cn = gates.tile([128, H], f32, tag="cn")
            iu = gates.tile([128, H], f32, tag="iu")
            nc.vector.tensor_tensor(iu[:PADl, :], i_t[:PADl, :],
                                    u_t[:PADl, :], op=mybir.AluOpType.mult)
            if G > 0:
                xf_bf = gates.tile([128, H], bf, tag="xfb")
                nc.vector.tensor_copy(xf_bf[:PADl, :], xg_l[:PADl, 3 * H:])
                f_t = work.tile([128, Gmax, H], bf, tag="f")
                for g in range(G):
                    pf = bp.tile([128, H], f32, tag="bp", space="PSUM")
                    for kk in range(HG):
                        nc.tensor.matmul(
                            pf[:], lhsT=chT[:, kk, g * 128:(g + 1) * 128],
                            rhs=w_fh[:, kk, :], start=(kk == 0), stop=False)
                    nc.tensor.matmul(
                        pf[:],
                        lhsT=t_PT[:PADl, poff + g * 128:poff + (g + 1) * 128],
                        rhs=xf_bf[:PADl, :], start=False, stop=True)
                    nc.scalar.activation(f_t[:, g, :], pf[:], SIG)
                fcc = work.tile([128, Gmax, H], bf, tag="fcc")
                nc.vector.tensor_tensor(fcc[:, 0:G, :], f_t[:, 0:G, :],
                                        cc[:, 0:G, :],
                                        op=mybir.AluOpType.mult)
                pcn = cnp.tile([128, H], f32, tag="cn", space="PSUM")
                for g in range(G):
                    nc.tensor.matmul(
                        pcn[:PADl, :],
                        lhsT=t_P[:, poff + g * 128:poff + g * 128 + PADl],
                        rhs=fcc[:, g, :], start=(g == 0), stop=(g == G - 1))
                nc.vector.tensor_tensor(cn[:PADl, :], pcn[:PADl, :],
                                        iu[:PADl, :],
                                        op=mybir.AluOpType.add)
            else:
                nc.vector.tensor_copy(cn[:PADl, :], iu[:PADl, :])

            tc_t = gates.tile([128, H], f32, tag="tc")
            nc.scalar.activation(tc_t[:PADl, :], cn[:PADl, :], TANH)
            h_f = gates.tile([128, H], f32, tag="hf")
            nc.vector.tensor_tensor(h_f[:PADl, :], o_t[:PADl, :],
                                    tc_t[:PADl, :], op=mybir.AluOpType.mult)

            delta = dpool.tile([128, 2 * H], bf, tag="delta")
            nc.gpsimd.memset(delta[:], 0.0)
            nc.vector.tensor_copy(delta[:PADl, 0:H], h_f[:PADl, :])
            nc.scalar.copy(delta[:PADl, H:2 * H], cn[:PADl, :])

            # per-row dynamic-range u8 quantization of h (scale shipped
            # separately): stored = floor(h*127/rowmax + 128.5)
            rm = gates.tile([128, 1], f32, tag="rm")
            nc.vector.tensor_reduce(out=rm[:PADl, :], in_=h_f[:PADl, :],
                                    axis=mybir.AxisListType.X,
                                    op=mybir.AluOpType.max,
                                    apply_absolute_value=True)
            nc.vector.tensor_scalar_max(out=rm[:PADl, :], in0=rm[:PADl, :],
                                        scalar1=1e-20)
            inv = gates.tile([128, 1], f32, tag="inv")
            nc.vector.reciprocal(inv[:PADl, :], rm[:PADl, :])
            nc.vector.tensor_scalar_mul(out=inv[:PADl, :],
                                        in0=inv[:PADl, :], scalar1=127.0)
            o_u8 = gates.tile([128, H], u8, tag="ou8")
            nc.vector.tensor_scalar(out=o_u8[:PADl, :], in0=h_f[:PADl, :],
                                    scalar1=inv[:PADl, :], scalar2=128.5,
                                    op0=mybir.AluOpType.mult,
                                    op1=mybir.AluOpType.add)
            sc = gates.tile([128, 1], f32, tag="sc")
            nc.vector.tensor_scalar_mul(out=sc[:PADl, :], in0=rm[:PADl, :],
                                        scalar1=1.0 / 127.0)
            nc.sync.dma_start(out=T_out[S_l:S_l + PADl, :],
                              in_=o_u8[:PADl, :])
            nc.sync.dma_start(out=T_os[S_l:S_l + PADl, :],
                              in_=sc[:PADl, :])
            ob = own_base + l * 128
            nc.sync.dma_start(
                out=T_log[2 * ob:2 * (ob + PADl), :].rearrange(
                    "(j two) h -> j (two h)", two=2),
                in_=delta[:PADl, :])
            if w_id < NW:
                woff = (meta_lvl[2 * w_id][0] if l == 2 * w_id + 1 else 0)
                nc.sync.dma_start(
                    out=T_ccin[w_id][woff:woff + PADl, :],
                    in_=delta[:PADl, :])
                if l == 2 * w_id + 1 and not NO_CC and w_id < CC_MAX:
                    nrows = C * WROWS[w_id]
                    nc.gpsimd.collective_compute(
                        "AllGather", mybir.AluOpType.bypass,
                        replica_groups=[list(range(C))],
                        ins=[T_ccin[w_id][:]],
                        outs=[T_ccout[w_id][:nrows, :]])
                    wb = winbase[w_id]
                    nc.sync.dma_start(
                        out=T_log[2 * wb:2 * (wb + nrows), :].rearrange(
                            "(j two) h -> j (two h)", two=2),
                        in_=T_ccout[w_id][:nrows, :])

    nc.compile()
    return nc
